# revision 21
# baseline (speedup 1.0000x reference)
"""Trainium2 Bass kernel for CapsuleLayer (dynamic routing), 8-core data-parallel.

Problem: x [128, 1152, 512] f32, W [512, 160] f32.
  u_hat = (x @ W).reshape(B, N, 10, 16)
  b = 0; 3 routing iterations of softmax/weighted-sum/squash.
Output: v [128, 10, 16] f32.

Sharding: data-parallel over batch. Each of the 8 cores gets 16 batches
(x shard [16*1152, 512]) and the full W; no cross-core communication.

Per-core pipeline (v2):
  Phase 1 (streamed over 144 row-tiles of 128):
    - DMA x tile f32 -> SBUF, cast to bf16 via SWDGE cast-DMA
    - transpose each [128,128] block on the PE as a regular bf16 matmul
      against an identity moving operand; TWO tiles of transposes share one
      PSUM buffer so the PSUM->SBUF copy (the expensive 1x f32 read) is
      batched and its fixed cost amortized
    - u_hat tile = xT.T @ W accumulated over the 4 k-chunks in PSUM,
      copied to SBUF as bf16 in [n, 160] layout (natural for routing)
    - all PSUM->SBUF copies are split between ACT and DVE by fixed ratios
  Phase 2 (on-chip routing, u_hat resident in SBUF):
    - capsule sums s via PE matmuls (c stationary), partition-broadcast via
      an all-ones stationary so squash needs no partition broadcast
    - softmax over capsules without max-subtraction (|b| stays small)
    - squash without Sqrt: factor = lam*sqrt(n2)/(1+lam^2 n2) computed as
      exp(0.5*ln(lam^2 n2) - ln(1+lam^2 n2) + ln lam), so the ONLY ACT
      table set used in the whole kernel is natural_log_exp (exp for
      softmax, ln/exp for squash, copy/square are filler in every set).
      The baseline's exp<->sqrt alternation cost 20 table loads (~26us).
    - iteration-0 capsule weight 1/C is folded into the squash factor
      (lam=0.1) so rt0 shares the all-ones stationary
"""

import math
import os
import sys

import numpy as np

sys.path.insert(0, "/opt/trn_rl_repo")

import concourse.bass as bass
import concourse.tile as tile
import concourse.mybir as mybir
from concourse import bacc

F32 = mybir.dt.float32
BF16 = mybir.dt.bfloat16
AF = mybir.ActivationFunctionType
ALU = mybir.AluOpType

B, N, K, C, D = 128, 1152, 512, 10, 16
CD = C * D  # 160
NCORES = 8
BSH = B // NCORES  # batches per core

XB_BATCH = 4  # n-tiles per x-load DMA
TP = 2  # transpose tiles per PSUM buffer (batched copy)
# PSUM->SBUF copy engine split patterns (cycled): True -> DVE, False -> ACT
XT_COPY_PAT = (False, False, False, True)  # transpose copies: 3/4 ACT, 1/4 DVE
# per transpose-PAIR: True -> DMA xbar transpose (Sync HWDGE queue, no PSUM
# round-trip, no copy), False -> PE identity-matmul path
XBAR_PAT = (False,)
UH_COPY_PAT = (False,)  # u_hat copies: all ACT


def build_core_program(bsh=BSH, n_len=N, nc=None):
    """Build the single-core Bass program for a shard of `bsh` batches."""
    assert n_len % 128 == 0
    tpb = n_len // 128  # row-tiles per batch
    rows = bsh * n_len

    if nc is None:
        nc = bacc.Bacc("TRN2", target_bir_lowering=False, debug=False)

    x_in = nc.declare_dram_parameter("x", [rows, K], F32, isOutput=False).ap()
    w_in = nc.declare_dram_parameter("W", [K, CD], F32, isOutput=False).ap()
    id_in = nc.declare_dram_parameter("ident", [128, 128], BF16, isOutput=False).ap()
    mk_in = nc.declare_dram_parameter("mask", [C, CD], BF16, isOutput=False).ap()
    v_out = nc.declare_dram_parameter("v", [bsh, CD], F32, isOutput=True).ap()

    with tile.TileContext(nc) as tc:
        _build_body(tc, x_in, w_in, id_in, mk_in, v_out, bsh, tpb)
    nc.finalize()
    return nc


def _build_body(tc, x_in, w_in, id_in, mk_in, v_out, bsh, tpb):
    nc = tc.nc
    nt = bsh * tpb
    KT = K // 128  # 4 contraction chunks

    from contextlib import ExitStack

    with ExitStack() as ctx:
        singles = ctx.enter_context(tc.tile_pool(name="singles", bufs=1))
        persist = ctx.enter_context(tc.tile_pool(name="persist", bufs=1))
        pool_xb = ctx.enter_context(tc.tile_pool(name="xb", bufs=3))
        pool_xT = ctx.enter_context(tc.tile_pool(name="xT", bufs=4))
        pool_sm = ctx.enter_context(tc.tile_pool(name="smalls", bufs=6))
        ps_U = ctx.enter_context(tc.tile_pool(name="psU", bufs=2, space="PSUM"))
        ps_P = ctx.enter_context(tc.tile_pool(name="psP", bufs=1, space="PSUM"))
        ps_S = ctx.enter_context(tc.tile_pool(name="psS", bufs=1, space="PSUM"))
        ps_T = ctx.enter_context(tc.tile_pool(name="psT", bufs=2, space="PSUM"))

        # --- x prefetch: issue the first loads before anything else so the
        # DMA latency overlaps the preamble and constant loads ---
        xb_pre = {}
        for tb in range(2):
            xb_t = pool_xb.tile([128, XB_BATCH, K], BF16, tag="xb", name="xb_pre")
            src_ap = x_in[tb * XB_BATCH * 128 : (tb + 1) * XB_BATCH * 128, :]
            nc.gpsimd.dma_start(
                out=xb_t, in_=src_ap.rearrange("(t p) k -> p t k", p=128)
            )
            xb_pre[tb] = xb_t

        # --- constants ---
        # Preload the one ACT table set the whole kernel uses
        # (natural_log_exp_and_others: Exp+Ln+Square+Copy), so the
        # auto-inserted per-function loads (which thrash between
        # exp_and_others and natural_log) are not needed.
        nc.scalar.add_instruction(
            mybir.InstLoadActFuncSet(
                name=nc.get_next_instruction_name(),
                act_func_set_id=6,
                engine=mybir.EngineType.Activation,
            )
        )
        ident = singles.tile([128, 128], BF16)
        nc.sync.dma_start(out=ident, in_=id_in)
        mask = singles.tile([C, CD], BF16)
        nc.sync.dma_start(out=mask, in_=mk_in)
        # all-ones stationary: capsule-sum matmuls use M=128 so the column
        # sums land replicated on all partitions, which lets squash and the
        # b-update run without any partition-broadcast (illegal on DVE).
        ones_m = singles.tile([128, 128], BF16)
        nc.vector.memset(ones_m, 1.0)
        lnl_bias = singles.tile([128, 1], F32)
        nc.vector.memset(lnl_bias, math.log(1.0 / C))

        w_f32 = singles.tile([128, KT, CD], F32)
        nc.sync.dma_start(out=w_f32, in_=w_in.rearrange("(j p) c -> p j c", p=128))
        w_bf = singles.tile([128, KT, CD], BF16)
        nc.vector.tensor_copy(w_bf, w_f32)

        # --- persistent tensors ---
        u_hat = persist.tile([128, nt, CD], BF16)
        w_scr = persist.tile([128, nt, CD], BF16)
        b_log = persist.tile([128, nt * C], F32)
        e_exp = persist.tile([128, nt * C], BF16)
        c_sm = persist.tile([128, nt * C], BF16)
        ssum = persist.tile([128, nt], F32)
        sq_all = persist.tile([128, bsh, CD], BF16)
        n2_all = persist.tile([128, bsh * C], F32)
        l1_all = persist.tile([128, bsh * C], F32)
        l2_all = persist.tile([128, bsh * C], F32)
        fc_all = persist.tile([128, bsh * C], BF16)
        vrep_bf = persist.tile([128, bsh, CD], BF16)
        vrep = persist.tile([128, bsh, CD], F32)

        # views
        u4 = u_hat[:].rearrange("p (g t) c -> p g t c", g=bsh)
        w4 = w_scr[:].rearrange("p (g t) c -> p g t c", g=bsh)
        w5 = w_scr[:].rearrange("p t (c d) -> p t c d", d=D)
        b3 = b_log[:].rearrange("p (t c) -> p t c", c=C)

        # ---------------- Phase 1 emitters ----------------
        # Transposes run on the PE as regular bf16 matmuls against the
        # identity; TP tiles (4 k-chunks each) share ONE psum buffer and get
        # one batched copy. The GEMM of pair m-1 is emitted after the
        # transposes of pair m so PE never stalls on the copy. u_hat
        # psum->SBUF copies are batched UCP tiles per copy.
        if bsh == 16:
            GSIZES = [1, 1, 2, 2, 2, 2, 2, 2, 2]
        else:
            GSIZES = [1] * bsh
        assert sum(GSIZES) == bsh
        GOFF = [sum(GSIZES[:k]) for k in range(len(GSIZES))]
        NGROUPS = len(GSIZES)
        HEAD_T = (GSIZES[0] + GSIZES[1]) * tpb if NGROUPS >= 2 else 0
        assert nt % XB_BATCH == 0
        xb_cur = [None]
        pt_cur = [None]
        xt_cur = [None]
        pu_cur = [None]
        copy_ctr = [0, 0]  # xt, uh

        def emit_trans(t):
            tb, tt = divmod(t, XB_BATCH)
            if tt == 0:
                if tb in xb_pre:
                    xb = xb_pre.pop(tb)
                else:
                    xb = pool_xb.tile([128, XB_BATCH, K], BF16, tag="xb")
                    src = x_in[tb * XB_BATCH * 128 : (tb + 1) * XB_BATCH * 128, :]
                    nc.gpsimd.dma_start(
                        out=xb, in_=src.rearrange("(t p) k -> p t k", p=128)
                    )
                xb_cur[0] = xb
            xb = xb_cur[0]
            pi = t % TP
            use_xbar = XBAR_PAT[(t // TP) % len(XBAR_PAT)]
            if use_xbar:
                if pi == 0:
                    xt_cur[0] = pool_xT.tile(
                        [128, TP, KT, 128], BF16, tag="xt", name="xtx"
                    )
                xt = xt_cur[0]
                nc.sync.dma_start_transpose(xt[:, pi, :, :], xb[:, tt, :])
                return xt if pi == TP - 1 else None
            if pi == 0:
                pt_cur[0] = ps_T.tile([128, TP, KT, 128], F32, tag="psT", name="pt")
            pt = pt_cur[0]
            for j in range(KT):
                nc.tensor.matmul(
                    pt[:, pi, j, :],
                    lhsT=xb[:, tt, j * 128 : (j + 1) * 128],
                    rhs=ident,
                    start=True,
                    stop=True,
                )
            if pi == TP - 1:
                xt = pool_xT.tile([128, TP, KT, 128], BF16, tag="xt")
                # head region (first two groups): DVE is idle while PE ramps,
                # so put all PSUM->SBUF copies there; later use the ACT-heavy
                # split so DVE can run the routing chains
                if t < HEAD_T:
                    use_dve = copy_ctr[0] % 2 == 0
                    copy_ctr[0] += 1
                else:
                    use_dve = XT_COPY_PAT[copy_ctr[0] % len(XT_COPY_PAT)]
                    copy_ctr[0] += 1
                if use_dve:
                    nc.vector.tensor_copy(xt, pt)
                else:
                    nc.scalar.copy(xt, pt)
                return xt
            return None

        UCP = 3 if tpb % 3 == 0 else (2 if tpb % 2 == 0 else 1)  # tiles per u-copy

        def emit_gemm(t, xt, slot):
            lt = t % tpb
            loc = lt % UCP
            if loc == 0 or pu_cur[0] is None:
                pu_cur[0] = ps_U.tile([128, UCP, CD], F32, tag="psU2", name="pu2")
            pu2 = pu_cur[0]
            for j in range(KT):
                nc.tensor.matmul(
                    pu2[:, loc, :],
                    lhsT=xt[:, slot, j, :],
                    rhs=w_bf[:, j, :],
                    start=(j == 0),
                    stop=(j == KT - 1),
                )
            fin = None
            if loc == UCP - 1:
                fin = (t - UCP + 1, t + 1, pu2)
            elif lt == tpb - 1:
                fin = (t - loc, t + 1, pu2[:, 0 : loc + 1, :])
            if fin is not None:
                lo, hi, src = fin
                if t < HEAD_T:
                    use_dve = copy_ctr[1] % 2 == 0
                    copy_ctr[1] += 1
                else:
                    use_dve = UH_COPY_PAT[copy_ctr[1] % len(UH_COPY_PAT)]
                    copy_ctr[1] += 1
                if use_dve:
                    nc.vector.tensor_copy(u_hat[:, lo:hi, :], src)
                else:
                    nc.scalar.copy(u_hat[:, lo:hi, :], src)
                pu_cur[0] = None

        # ---------------- Phase 2 (routing) emitters, per group ----------

        def squash_group(g0, gb, i, sp):
            # v = squash(lam*s) with s the RAW capsule sums (lam folds the
            # uniform softmax weight 1/C of iteration 0).
            # factor = lam*sqrt(n2)/(1+lam^2 n2)
            #        = exp(0.5*ln(lam^2 n2) - ln(1+lam^2 n2) + ln lam)
            # (the +1e-7 of the reference only perturbs v by O(1e-7), so it
            # is dropped; n2=0 degrades gracefully: ln->-inf, exp->0)
            last = i == 2
            lam2 = (1.0 / C) ** 2 if i == 0 else 1.0
            lbias = lnl_bias[:, :] if i == 0 else 0.0
            gs = slice(g0, g0 + gb)
            cs = slice(g0 * C, (g0 + gb) * C)
            s_g = sp[:, :, 0:CD]
            sq_g = sq_all[:, gs, :]
            n2_g = n2_all[:, cs]
            l1_g = l1_all[:, cs]
            l2_g = l2_all[:, cs]
            fc_g = fc_all[:, cs]
            nc.scalar.activation(sq_g, s_g, AF.Square)
            nc.vector.tensor_reduce(
                n2_g,
                sq_g.rearrange("p g (c d) -> p (g c) d", d=D),
                axis=mybir.AxisListType.X,
                op=ALU.add,
            )
            nc.scalar.activation(l1_g, n2_g, AF.Ln, scale=lam2)
            nc.scalar.activation(l2_g, n2_g, AF.Ln, scale=lam2, bias=1.0)
            nc.vector.scalar_tensor_tensor(
                l1_g, l1_g, 0.5, l2_g, op0=ALU.mult, op1=ALU.subtract
            )
            nc.scalar.activation(fc_g, l1_g, AF.Exp, bias=lbias)
            fb = fc_g.rearrange("p (g c) -> p g c", c=C).broadcast_to(
                [128, gb, C, D]
            )
            out = (vrep if last else vrep_bf)[:, gs, :]
            nc.vector.tensor_mul(
                out.rearrange("p g (c d) -> p g c d", d=D),
                s_g.rearrange("p g (c d) -> p g c d", d=D),
                fb,
            )
            if last:
                nc.sync.dma_start(
                    out=v_out[g0 : g0 + gb, :], in_=vrep[0:1, gs, :]
                )

        def rt0(g0, gb):
            # s0_raw = sum_{n,t} u_hat per batch, on the PE (1/C folded into
            # squash's lam); squash reads the PSUM group tile directly
            sp = ps_S.tile([128, gb, 256], F32, tag="psS", name="sp")
            for gi in range(gb):
                g = g0 + gi
                for tt in range(tpb):
                    t = g * tpb + tt
                    nc.tensor.matmul(
                        sp[:, gi, 0:CD],
                        lhsT=ones_m,
                        rhs=u_hat[:, t, :],
                        start=(tt == 0),
                        stop=(tt == tpb - 1),
                    )
            squash_group(g0, gb, 0, sp)

        def rt12_a(g0, gb, i):
            gs = slice(g0, g0 + gb)
            ts = slice(g0 * tpb, (g0 + gb) * tpb)
            tg = gb * tpb
            # b update: b (+)= sum_d u_hat * v_prev  (bf16 2x tree adds)
            vb = (
                vrep_bf[:, gs, :]
                .broadcast_to([128, gb, CD, tpb])
                .rearrange("p g c t -> p g t c")
            )
            nc.vector.tensor_mul(w4[:, gs, :, :], u4[:, gs, :, :], vb)
            wg = w5[:, ts, :, :]
            nc.vector.tensor_add(wg[:, :, :, 0:8], wg[:, :, :, 0:8], wg[:, :, :, 8:16])
            nc.vector.tensor_add(wg[:, :, :, 0:4], wg[:, :, :, 0:4], wg[:, :, :, 4:8])
            nc.vector.tensor_add(wg[:, :, :, 0:2], wg[:, :, :, 0:2], wg[:, :, :, 2:4])
            bg = b3[:, ts, :]
            if i == 1:
                nc.vector.tensor_add(bg, wg[:, :, :, 0], wg[:, :, :, 1])
            else:
                nc.vector.tensor_add(wg[:, :, :, 0], wg[:, :, :, 0], wg[:, :, :, 1])
                nc.vector.tensor_add(bg, bg, wg[:, :, :, 0])
            # softmax over capsules (no max-subtraction: |b| is small)
            fs = slice(g0 * tpb * C, (g0 + gb) * tpb * C)
            e_g = e_exp[:, fs]
            nc.scalar.activation(e_g, b_log[:, fs], AF.Exp)
            ss_g = ssum[:, ts]
            nc.vector.tensor_reduce(
                ss_g,
                e_g.rearrange("p (t c) -> p t c", c=C),
                axis=mybir.AxisListType.X,
                op=ALU.add,
            )
            nc.vector.reciprocal(ss_g, ss_g)
            rb = ss_g.broadcast_to([128, tg, C])
            c_g = c_sm[:, fs]
            nc.vector.tensor_mul(
                c_g.rearrange("p (t c) -> p t c", c=C),
                e_g.rearrange("p (t c) -> p t c", c=C),
                rb,
            )

        def rt12_b(g0, gb, i):
            # s[c,d] = sum_n c*u via per-tile matmuls with c stationary;
            # squash reads the PSUM group tile directly
            sp = ps_S.tile([128, gb, 256], F32, tag="psS", name="sp")
            for gi in range(gb):
                g = g0 + gi
                pp = ps_P.tile([C, CD], F32, tag="psP", name="pp")
                for tt in range(tpb):
                    t = g * tpb + tt
                    nc.tensor.matmul(
                        pp,
                        lhsT=c_sm[:, t * C : (t + 1) * C],
                        rhs=u_hat[:, t, :],
                        start=(tt == 0),
                        stop=(tt == tpb - 1),
                    )
                pm = pool_sm.tile([C, CD], BF16, tag="pm")
                nc.vector.tensor_mul(pm, pp, mask)
                nc.tensor.matmul(
                    sp[:, gi, 0:CD], lhsT=ones_m[0:C, :], rhs=pm, start=True, stop=True
                )
            squash_group(g0, gb, i, sp)

        # ---------------- interleaved emission ----------------
        # Continuous tile stream (pairs may span group boundaries); the
        # GEMM of pair m-1 is emitted after the transposes of pair m.
        # When a group's last GEMM is out, fire its routing slot.
        def fire(k):
            g0, gb = GOFF[k], GSIZES[k]
            if k >= 1:
                rt12_a(GOFF[k - 1], GSIZES[k - 1], 1)
            if k >= 2:
                rt12_a(GOFF[k - 2], GSIZES[k - 2], 2)
            rt0(g0, gb)
            if k >= 1:
                rt12_b(GOFF[k - 1], GSIZES[k - 1], 1)
            if k >= 2:
                rt12_b(GOFF[k - 2], GSIZES[k - 2], 2)

        gemmed = -1  # last tile whose GEMM has been emitted
        nextg = 0  # next group to fire
        prevp = None

        def flush_groups():
            nonlocal nextg
            while nextg < NGROUPS and (GOFF[nextg] + GSIZES[nextg]) * tpb - 1 <= gemmed:
                fire(nextg)
                nextg += 1

        for t in range(nt):
            xt = emit_trans(t)
            if xt is not None:
                if prevp is not None:
                    pt0, pxt = prevp
                    emit_gemm(pt0, pxt, 0)
                    emit_gemm(pt0 + 1, pxt, 1)
                    gemmed = pt0 + 1
                    flush_groups()
                prevp = (t - TP + 1, xt)
        pt0, pxt = prevp
        emit_gemm(pt0, pxt, 0)
        emit_gemm(pt0 + 1, pxt, 1)
        gemmed = pt0 + 1
        flush_groups()
        assert nextg == NGROUPS
        KG = NGROUPS
        rt12_a(GOFF[KG - 1], GSIZES[KG - 1], 1)
        rt12_b(GOFF[KG - 1], GSIZES[KG - 1], 1)
        if KG >= 2:
            rt12_a(GOFF[KG - 2], GSIZES[KG - 2], 2)
            rt12_b(GOFF[KG - 2], GSIZES[KG - 2], 2)
        rt12_a(GOFF[KG - 1], GSIZES[KG - 1], 2)
        rt12_b(GOFF[KG - 1], GSIZES[KG - 1], 2)


# ----------------------------------------------------------------------------
_NC_CACHE = {}


def _get_nc():
    key = (BSH, N)
    if key not in _NC_CACHE:
        _NC_CACHE[key] = build_core_program()
    return _NC_CACHE[key]


def _run(x, W, **kw):
    from concourse.bass_utils import run_bass_kernel_spmd

    import ml_dtypes

    nc = _get_nc()
    x = np.ascontiguousarray(x, dtype=np.float32)
    W = np.ascontiguousarray(W, dtype=np.float32)
    ident = np.eye(128, dtype=ml_dtypes.bfloat16)
    mask = np.kron(np.eye(C, dtype=np.float32), np.ones((1, D), np.float32)).astype(
        ml_dtypes.bfloat16
    )
    shards = x.reshape(NCORES, BSH * N, K)
    in_maps = [
        {"x": shards[c], "W": W, "ident": ident, "mask": mask} for c in range(NCORES)
    ]
    res = run_bass_kernel_spmd(nc, in_maps, core_ids=list(range(NCORES)), **kw)
    v = np.concatenate(
        [res.results[c]["v"].reshape(BSH, C, D) for c in range(NCORES)], axis=0
    )
    return v, res


def kernel(x, W):
    v, _ = _run(x, W)
    return v


def kernel_timed(x, W):
    v, res = _run(x, W, trace=True)
    return v, res.exec_time_ns


def kernel_traced(x, W):
    v, res = _run(x, W, trace=True)
    return v, res
